# revision 26
# baseline (speedup 1.0000x reference)
"""Trainium2 Bass kernel for nn_DynamicDictionaryLearning (vq_codebook).

Computation (full shapes):
    query_embed = (basic_queries @ W_mlp + b_mlp).reshape(T, R, D)    # (T, R*D)
    dynamic_queries = einsum('btr,trd->btd', query_weights, query_embed)
    basic_expanded  = broadcast(basic_queries, (B, T, D))

Sharding (8 NeuronCores, one chip):
    Stage 1 (token-MLP expansion) is tensor-sharded over the R*D output
    dim: core r computes qe_r = basic_queries @ W_mlp[:, r*D:(r+1)*D] +
    b_r for ALL tokens, as 512-col k-outer passes (one PSUM bank per
    token-tile accumulation chain).  Each 256-col chunk is AllToAll'd as
    soon as its slice lands in DRAM; stage-2 chunks (weighted sum over R
    as dense PE matmuls against block-diagonal qw tiles) are interleaved
    into the PE stream one pass behind, so the collectives and both DMA
    rings pipeline behind the PE.

    Schedule notes: W loads are pass-column-sliced so the first pass is
    compute-paced rather than blocked on the full 8.4MB W stream; bq
    rides the SP ring, W the ACT ring.  qe->ain stores are split into
    two half-stores (one per ring), issued as soon as their 4 bias-adds
    retire.  q2 gathers are emitted inside the producing pass so they
    never queue behind a later pass's stores.  bq tokens are host-side
    swizzled (p' = tt*8+g per 128-token tile) which makes each gather a
    single flat contiguous DMA.  PSUM->SBUF casts all run on DVE; dq
    stores are merged per token-group pair.  A full-size dependency-free
    warmup collective absorbs CC bring-up and NEFF start skew.  All
    matmul operands and the dq store are bf16 (fp32 PSUM accumulation).

    basic_expanded is a pure broadcast of an input -> host-side view.
"""

import numpy as np
import ml_dtypes

import concourse.bass as bass
import concourse.mybir as mybir
import concourse.tile as tile
from concourse import bacc
from concourse.bass_utils import run_bass_kernel_spmd

# Problem shapes (hardcoded per spec)
D = 2048
T = 1024
R = 8
B = 32
NC = 8
TS = T // NC          # 128 tokens per core (stage-2 ownership)
P = 128
KT = D // P           # 16 contraction tiles
MT = T // P           # 8 token tiles (stage 1)
NQ = 9                # a2a chunks (D-slices), last two half-width
NW = 256              # nominal chunk width
TG = TS // 16         # 8 token groups of 16 (stage 2)
BG = B // 8           # 4 batch groups of 8 (stage 2)
PSW = 1024            # PSUM slot width (2 banks); 4 slots = all 8 banks

F32 = mybir.dt.float32
BF16 = mybir.dt.bfloat16
DT_MM = BF16
NP_MM = ml_dtypes.bfloat16

# stage-1 passes: (col_lo, width).  The last 512 cols split so late
# chunks finish staggered and their AllToAlls don't serialize.
PASSES = [(0, 512), (512, 512), (1024, 512), (1536, 256), (1792, 256)]
# a2a chunks: (col_lo, width) -- last two are half-width so the final
# collective is small and its latency fits under the s2 tail
CHUNKS = [(0, 256), (256, 256), (512, 256), (768, 256), (1024, 256),
          (1280, 256), (1536, 256), (1792, 128), (1920, 128)]
# W load slabs: (col band, k group) -> one tile each
WGRP = [(0, 512), (512, 512), (1024, 512), (1536, 512)]
KG = 4                # k-tiles per W slab
# s2 chunks emitted after each pass index (PE program order); tail-heavy
# so post-final-pass PE work covers the last AllToAll's latency, and a
# two-pass lag so no s2 ever waits on collective completion mid-stream
S2_AFTER = {2: [0, 1], 3: [2], 4: [3, 4, 5, 6, 7, 8]}

_cache = {}


def _build_nc():
    nc = bacc.Bacc("TRN2", target_bir_lowering=False, num_devices=NC)

    bqT = nc.dram_tensor("bqT", [D, T], DT_MM, kind="ExternalInput")
    Wc = nc.dram_tensor("Wc", [D, D], DT_MM, kind="ExternalInput")
    biasr = nc.dram_tensor("biasr", [P, D], F32, kind="ExternalInput")
    # block-diagonal qw tiles, packed (128, 32*128) for one big-line DMA
    Lt = nc.dram_tensor("Lt", [P, TG * BG * P], DT_MM, kind="ExternalInput")
    # flat per-(chunk, token-group-pair) contiguous blocks; host reassembles
    dq = nc.dram_tensor("dq", [B * TS * D], BF16, kind="ExternalOutput")
    dq_base = np.cumsum([0] + [TG // 2 * P * 2 * BG * w for _, w in CHUNKS])

    bqT_t = bqT.rearrange("(kt p) m -> kt p m", p=P)   # (16, 128, 1024)

    with tile.TileContext(nc) as tc:
        with (
            tc.tile_pool(name="bqp", bufs=1) as bqpool,
            tc.tile_pool(name="wp", bufs=1) as wpool,
            tc.tile_pool(name="constp", bufs=1) as cpool,
            tc.tile_pool(name="qep", bufs=3) as qepool,
            tc.tile_pool(name="q2p", bufs=5) as q2pool,
            tc.tile_pool(name="o2p", bufs=3) as o2pool,
            tc.tile_pool(name="psp", bufs=4, space="PSUM") as pspool,
            tc.tile_pool(name="dramp", bufs=1, space="DRAM") as dram,
        ):
            # warmup collective, full chunk size: absorbs CC bring-up +
            # NEFF start skew while the input loads stream
            wdum = dram.tile([P, MT * NW], DT_MM, name="wdum")
            adum = dram.tile([P, MT * NW], DT_MM, name="adum")
            nc.gpsimd.collective_compute(
                "AllToAll",
                mybir.AluOpType.bypass,
                replica_groups=[list(range(NC))],
                ins=[wdum.opt()],
                outs=[adum.opt()],
            )

            # bq k-tiles + bias on SP ring
            bq_tiles = []
            for k in range(KT):
                bt = bqpool.tile([P, T], DT_MM, name=f"bq{k}")
                nc.sync.dma_start(out=bt, in_=bqT_t[k])
                bq_tiles.append(bt)
            bias_t = cpool.tile([P, D], F32, name="bias")
            nc.sync.dma_start(out=bias_t, in_=biasr[:, :])
            # W on ACT ring in (col band, 4-k group) slabs: pass 0 is
            # compute-paced (band 0 streams in 4-k granularity) and the
            # ACT ring stays issue-light for the ain half-stores
            Wc_v = Wc.rearrange("(k p) d -> p k d", p=P)
            w_slabs = {}
            for ci, (lo, w) in enumerate(WGRP):
                kg_n = 2 if ci == 0 else KG  # finer pacing for pass 0
                for kg in range(KT // kg_n):
                    wt = wpool.tile([P, kg_n * w], DT_MM, name=f"w{ci}_{kg}")
                    nc.scalar.dma_start(
                        out=wt,
                        in_=Wc_v[:, kg * kg_n:(kg + 1) * kg_n, lo:lo + w],
                    )
                    w_slabs[(ci, kg)] = wt
                if ci == 1:
                    lbig = cpool.tile([P, TG * BG * P], DT_MM, name="lbig")
                    nc.scalar.dma_start(out=lbig, in_=Lt[:, :])

            def w_slice(ci, k, woff, w):
                kg_n = 2 if ci == 0 else KG
                slab = w_slabs[(ci, k // kg_n)]
                base = (k % kg_n) * WGRP[ci][1]
                return slab[:, base + woff:base + woff + w]
            l_tiles = {
                (g, h): lbig[:, (g * BG + h) * P:(g * BG + h + 1) * P]
                for g in range(TG)
                for h in range(BG)
            }

            # token swizzle v2 (see _prep_inputs): qe partition p=(c8,u),
            # col block mt=token-group -> ain[j] is a plain contiguous
            # (128, MT*w) store, the a2a chunk boundary is a 16-partition
            # row band, and aout IS the q2 layout (flat gather).
            ain = [dram.tile([P, MT * w], DT_MM, name=f"ain{j}")
                   for j, (_, w) in enumerate(CHUNKS)]
            aout = [dram.tile([P, MT * w], DT_MM, name=f"aout{j}")
                    for j, (_, w) in enumerate(CHUNKS)]
            q2_tiles = {}

            def a2a(j):
                nc.gpsimd.collective_compute(
                    "AllToAll",
                    mybir.AluOpType.bypass,
                    replica_groups=[list(range(NC))],
                    ins=[ain[j].opt()],
                    outs=[aout[j].opt()],
                )

            def stage1_pass(pi):
                lo, w = PASSES[pi]
                chunks = [j for j, (clo, cw) in enumerate(CHUNKS)
                          if lo <= clo < lo + w]
                ci = lo // 512
                woff = lo - WGRP[ci][0]
                with nc.named_scope(f"s1_p{pi}"):
                    ps = [pspool.tile([P, PSW], F32, name="ps")
                          for _ in range(4)]

                    def chain(m):
                        # one accumulation chain per 2KB PSUM bank:
                        # start=True clears the whole bank's has_written
                        # bits, so chains must never share a bank
                        return ps[m // 2][:, (m % 2) * 512:(m % 2) * 512 + w]

                    for k in range(KT):
                        for m in range(MT):
                            nc.tensor.matmul(
                                chain(m),
                                bq_tiles[k][:, m * P:(m + 1) * P],
                                w_slice(ci, k, woff, w),
                                start=(k == 0),
                                stop=(k == KT - 1),
                            )
                    # bias-adds write per-chunk qe tiles so the ain store
                    # is a plain contiguous 2D->2D copy; chunk-major add
                    # order lets the first chunk's a2a trigger early
                    for j in chunks:
                        clo, cw = CHUNKS[j]
                        off = clo - lo
                        qe = qepool.tile([P, MT * cw], DT_MM, name="qe")
                        for m in range(MT):
                            nc.vector.tensor_add(
                                qe[:, m * cw:(m + 1) * cw],
                                chain(m)[:, off:off + cw],
                                bias_t[:, clo:clo + cw],
                            )
                        # halved across the two HWDGE rings by partition
                        nc.sync.dma_start(
                            out=ain[j][0:P // 2, :],
                            in_=qe[0:P // 2, :],
                        )
                        nc.scalar.dma_start(
                            out=ain[j][P // 2:P, :],
                            in_=qe[P // 2:P, :],
                        )
                        a2a(j)
                    # gathers for this pass's chunks: emitted here so they
                    # never queue behind a later pass's stores.  bq tokens
                    # are host-swizzled so this is one flat contiguous DMA.
                    for j in chunks:
                        cw = CHUNKS[j][1]
                        q2 = q2pool.tile([P, TG * cw], DT_MM, name="q2")
                        nc.sync.dma_start(out=q2[:, :], in_=aout[j][:, :])
                        q2_tiles[j] = q2

            def stage2(j):
                cw = CHUNKS[j][1]
                with nc.named_scope(f"s2_q{j}"):
                    q2 = q2_tiles.pop(j)
                    for gp in range(TG // 2):
                        o2 = o2pool.tile([P, 2 * BG * cw], BF16, name="o2")
                        for gi in range(2):
                            g = gp * 2 + gi
                            # 4 batch-group matmuls -> one PSUM tile ->
                            # one wide cast (DVE/ACT alternating so casts
                            # keep pace with the PE in the tail)
                            ps2 = pspool.tile([P, PSW], F32, name="ps")
                            for h in range(BG):
                                nc.tensor.matmul(
                                    ps2[:, h * cw:(h + 1) * cw],
                                    l_tiles[(g, h)][:, :],
                                    q2[:, g * cw:(g + 1) * cw],
                                    start=True,
                                    stop=True,
                                )
                            dst = o2[:, gi * BG * cw:(gi + 1) * BG * cw]
                            if gi == 0:
                                nc.vector.tensor_copy(
                                    dst, ps2[:, :BG * cw])
                            else:
                                nc.scalar.copy(dst, ps2[:, :BG * cw])
                        sz = P * 2 * BG * cw
                        off = int(dq_base[j]) + gp * sz
                        if gp % 2 == 0:
                            nc.sync.dma_start(out=dq[off:off + sz],
                                              in_=o2[:, :])
                        else:
                            nc.scalar.dma_start(out=dq[off:off + sz],
                                                in_=o2[:, :])

            for pi in range(len(PASSES)):
                stage1_pass(pi)
                for j in S2_AFTER.get(pi, []):
                    stage2(j)

    nc.finalize()
    return nc


def _prep_inputs(query_weights, basic_queries, W_mlp, b_mlp):
    qw = np.ascontiguousarray(query_weights, dtype=np.float32)
    bq = np.ascontiguousarray(basic_queries, dtype=np.float32)
    W = np.ascontiguousarray(W_mlp, dtype=np.float32)
    b = np.ascontiguousarray(b_mlp, dtype=np.float32)

    # token swizzle v2: bqT column (mt*128 + c8*16 + u) holds token
    # (c8*128 + mt*16 + u), i.e. stage-1 M-tile mt = token group, qe
    # partition p = (dest core c8, in-group index u).  This makes the
    # qe->ain store contiguous, the a2a chunk boundary a partition band,
    # and aout exactly the stage-2 q2 layout.
    mt_i = np.arange(MT)[:, None, None]
    c8_i = np.arange(NC)[None, :, None]
    u_i = np.arange(16)[None, None, :]
    tok_idx = (c8_i * P + mt_i * 16 + u_i).reshape(-1)  # new-col -> old token
    bqT = np.ascontiguousarray(bq.T[:, tok_idx].astype(NP_MM))  # (D, T)

    g_i = np.arange(TG)[:, None, None, None, None]
    h_i = np.arange(BG)[None, :, None, None, None]
    tt_i = np.arange(16)[None, None, :, None, None]
    r_i = np.arange(R)[None, None, None, :, None]
    bb_i = np.arange(8)[None, None, None, None, :]

    in_maps = []
    for c in range(NC):
        Wc = np.ascontiguousarray(W[:, c * D:(c + 1) * D].astype(NP_MM))
        biasr = np.ascontiguousarray(
            np.broadcast_to(b[c * D:(c + 1) * D], (P, D))
        )
        qw_c = qw[:, c * TS:(c + 1) * TS, :]  # (32, 128, 8)
        # K index r*16+tt (r-major), M index bb*16+tt (b-major)
        L = np.zeros((TG, BG, P, P), NP_MM)
        L[g_i, h_i, r_i * 16 + tt_i, bb_i * 16 + tt_i] = \
            qw_c[h_i * 8 + bb_i, g_i * 16 + tt_i, r_i].astype(NP_MM)
        # pack to (128, 32*128): Lbig[p, (g*BG+h)*128 + m] = L[g, h, p, m]
        Lbig = np.ascontiguousarray(
            L.transpose(2, 0, 1, 3).reshape(P, TG * BG * P)
        )
        in_maps.append({"bqT": bqT, "Wc": Wc, "biasr": biasr, "Lt": Lbig})
    return in_maps


last_results = None  # exposed for external profiling harnesses


def kernel(query_weights, basic_queries, W_mlp, b_mlp):
    global last_results
    if "nc" not in _cache:
        _cache["nc"] = _build_nc()
    nc = _cache["nc"]

    in_maps = _prep_inputs(query_weights, basic_queries, W_mlp, b_mlp)
    res = run_bass_kernel_spmd(nc, in_maps, core_ids=list(range(NC)))
    last_results = res

    # dq flat blocks [j][gp][(bb,tt)][(gi,h,c)] -> (B, TS, D):
    # b = h*8+bb, t = (2*gp+gi)*16+tt, d = chunk_lo + c
    base = np.cumsum([0] + [TG // 2 * P * 2 * BG * w for _, w in CHUNKS])
    parts = []
    for c in range(NC):
        flat = res.results[c]["dq"]
        dcs = []
        for j, (clo, cw) in enumerate(CHUNKS):
            arr = flat[base[j]:base[j + 1]].reshape(TG // 2, 8, 16, 2, BG, cw)
            # (gp, bb, tt, gi, h, c) -> (h, bb, gp, gi, tt, c)
            dcs.append(arr.transpose(4, 1, 0, 3, 2, 5).reshape(B, TS, cw))
        parts.append(np.concatenate(dcs, axis=2).astype(np.float32))
    dq_full = np.concatenate(parts, axis=1)
    basic_expanded = np.broadcast_to(
        np.ascontiguousarray(basic_queries, dtype=np.float32)[None], (B, T, D)
    )
    return dq_full, basic_expanded


# revision 28
# speedup vs baseline: 1.0960x; 1.0960x over previous
"""Trainium2 Bass kernel for nn_DynamicDictionaryLearning (vq_codebook).

Computation (full shapes):
    query_embed = (basic_queries @ W_mlp + b_mlp).reshape(T, R, D)    # (T, R*D)
    dynamic_queries = einsum('btr,trd->btd', query_weights, query_embed)
    basic_expanded  = broadcast(basic_queries, (B, T, D))

Sharding (8 NeuronCores, one chip):
    Stage 1 (token-MLP expansion) is tensor-sharded over the R*D output
    dim: core r computes qe_r = basic_queries @ W_mlp[:, r*D:(r+1)*D] +
    b_r for ALL tokens, as 512-col k-outer passes (one PSUM bank per
    token-tile accumulation chain).  Each 256-col chunk is AllToAll'd as
    soon as its slice lands in DRAM; stage-2 chunks (weighted sum over R
    as dense PE matmuls against block-diagonal qw tiles) are interleaved
    into the PE stream one pass behind, so the collectives and both DMA
    rings pipeline behind the PE.

    Schedule notes: W loads are pass-column-sliced so the first pass is
    compute-paced rather than blocked on the full 8.4MB W stream; bq
    rides the SP ring, W the ACT ring.  qe->ain stores are split into
    two half-stores (one per ring), issued as soon as their 4 bias-adds
    retire.  q2 gathers are emitted inside the producing pass so they
    never queue behind a later pass's stores.  bq tokens are host-side
    swizzled (p' = tt*8+g per 128-token tile) which makes each gather a
    single flat contiguous DMA.  PSUM->SBUF casts all run on DVE; dq
    stores are merged per token-group pair.  A full-size dependency-free
    warmup collective absorbs CC bring-up and NEFF start skew.  All
    matmul operands and the dq store are bf16 (fp32 PSUM accumulation).

    basic_expanded is a pure broadcast of an input -> host-side view.
"""

import numpy as np
import ml_dtypes

import concourse.bass as bass
import concourse.mybir as mybir
import concourse.tile as tile
from concourse import bacc
from concourse.bass_utils import run_bass_kernel_spmd

# Problem shapes (hardcoded per spec)
D = 2048
T = 1024
R = 8
B = 32
NC = 8
TS = T // NC          # 128 tokens per core (stage-2 ownership)
P = 128
KT = D // P           # 16 contraction tiles
MT = T // P           # 8 token tiles (stage 1)
NQ = 8                # a2a chunks (D-slices)
NW = D // NQ          # 256 cols per chunk
TG = TS // 16         # 8 token groups of 16 (stage 2)
BG = B // 8           # 4 batch groups of 8 (stage 2)
PSW = 512             # PSUM slot width: 1 bank x 8 slots = all 8 banks

F32 = mybir.dt.float32
BF16 = mybir.dt.bfloat16
DT_MM = BF16
NP_MM = ml_dtypes.bfloat16

# stage-1 passes: (col_lo, width).  The last 512 cols split in two so
# chunks 6/7 finish staggered and their AllToAlls don't serialize.
PASSES = [(0, 512), (512, 512), (1024, 512), (1536, 256), (1792, 256)]
# W load slabs: (col band, 4-k group) -> one (128, 4*512) tile each
WGRP = [(0, 512), (512, 512), (1024, 512), (1536, 512)]
KG = 4                # k-tiles per W slab
# s2 chunks emitted after each pass index (PE program order); tail-heavy
# so post-final-pass PE work covers the last AllToAll's latency, and a
# two-pass lag so no s2 ever waits on collective completion mid-stream
S2_AFTER = {2: [0, 1], 3: [2], 4: [3, 4, 5, 6, 7]}

_cache = {}


def _build_nc():
    nc = bacc.Bacc("TRN2", target_bir_lowering=False, num_devices=NC)

    bqT = nc.dram_tensor("bqT", [D, T], DT_MM, kind="ExternalInput")
    Wc = nc.dram_tensor("Wc", [D, D], DT_MM, kind="ExternalInput")
    biasr = nc.dram_tensor("biasr", [P, D], F32, kind="ExternalInput")
    # block-diagonal qw tiles, packed (128, 32*128) for one big-line DMA
    Lt = nc.dram_tensor("Lt", [P, TG * BG * P], DT_MM, kind="ExternalInput")
    # per-(chunk, token-group-pair) contiguous blocks; host reassembles
    dq = nc.dram_tensor("dq", [NQ, TG // 2, P, 2 * BG * NW], BF16,
                        kind="ExternalOutput")

    bqT_t = bqT.rearrange("(kt p) m -> kt p m", p=P)   # (16, 128, 1024)

    with tile.TileContext(nc) as tc:
        with (
            tc.tile_pool(name="bqp", bufs=1) as bqpool,
            tc.tile_pool(name="wp", bufs=1) as wpool,
            tc.tile_pool(name="constp", bufs=1) as cpool,
            tc.tile_pool(name="qep", bufs=2) as qepool,
            tc.tile_pool(name="q2p", bufs=5) as q2pool,
            tc.tile_pool(name="o2p", bufs=4) as o2pool,
            tc.tile_pool(name="psp", bufs=8, space="PSUM") as pspool,
            tc.tile_pool(name="dramp", bufs=1, space="DRAM") as dram,
        ):
            # warmup collective, full chunk size: absorbs CC bring-up +
            # NEFF start skew while the input loads stream
            wdum = dram.tile([P, MT * NW], DT_MM, name="wdum")
            adum = dram.tile([P, MT * NW], DT_MM, name="adum")
            nc.gpsimd.collective_compute(
                "AllToAll",
                mybir.AluOpType.bypass,
                replica_groups=[list(range(NC))],
                ins=[wdum.opt()],
                outs=[adum.opt()],
            )

            # bq k-tiles + bias on SP ring
            bq_tiles = []
            for k in range(KT):
                bt = bqpool.tile([P, T], DT_MM, name=f"bq{k}")
                nc.sync.dma_start(out=bt, in_=bqT_t[k])
                bq_tiles.append(bt)
            bias_t = cpool.tile([P, D], F32, name="bias")
            nc.sync.dma_start(out=bias_t, in_=biasr[:, :])
            # W on ACT ring in (col band, 4-k group) slabs: pass 0 is
            # compute-paced (band 0 streams in 4-k granularity) and the
            # ACT ring stays issue-light for the ain half-stores
            Wc_v = Wc.rearrange("(k p) d -> p k d", p=P)
            w_slabs = {}
            for ci, (lo, w) in enumerate(WGRP):
                kg_n = 2 if ci == 0 else KG  # finer pacing for pass 0
                for kg in range(KT // kg_n):
                    wt = wpool.tile([P, kg_n * w], DT_MM, name=f"w{ci}_{kg}")
                    nc.scalar.dma_start(
                        out=wt,
                        in_=Wc_v[:, kg * kg_n:(kg + 1) * kg_n, lo:lo + w],
                    )
                    w_slabs[(ci, kg)] = wt
                if ci == 1:
                    lbig = cpool.tile([P, TG * BG * P], DT_MM, name="lbig")
                    nc.scalar.dma_start(out=lbig, in_=Lt[:, :])

            def w_slice(ci, k, woff, w):
                kg_n = 2 if ci == 0 else KG
                slab = w_slabs[(ci, k // kg_n)]
                base = (k % kg_n) * WGRP[ci][1]
                return slab[:, base + woff:base + woff + w]
            l_tiles = {
                (g, h): lbig[:, (g * BG + h) * P:(g * BG + h + 1) * P]
                for g in range(TG)
                for h in range(BG)
            }

            # token swizzle v2 (see _prep_inputs): qe partition p=(c8,u),
            # col block mt=token-group -> ain[j] is a plain contiguous
            # (128, MT*NW) store, the a2a chunk boundary is a 16-partition
            # row band, and aout IS the q2 layout (flat gather).
            ain = [dram.tile([P, MT * NW], DT_MM, name=f"ain{j}")
                   for j in range(NQ)]
            aout = [dram.tile([P, MT * NW], DT_MM, name=f"aout{j}")
                    for j in range(NQ)]
            q2_tiles = {}

            def a2a(j):
                nc.gpsimd.collective_compute(
                    "AllToAll",
                    mybir.AluOpType.bypass,
                    replica_groups=[list(range(NC))],
                    ins=[ain[j].opt()],
                    outs=[aout[j].opt()],
                )

            def stage1_pass(pi):
                lo, w = PASSES[pi]
                chunks = list(range(lo // NW, (lo + w) // NW))
                ci = lo // 512
                woff = lo - WGRP[ci][0]
                with nc.named_scope(f"s1_p{pi}"):
                    qe = qepool.tile([P, MT * w], DT_MM, name="qe")
                    ps = [pspool.tile([P, PSW], F32, name="ps")
                          for _ in range(MT)]

                    def chain(m):
                        # one accumulation chain per 2KB PSUM bank:
                        # start=True clears the whole bank's has_written
                        # bits, so chains must never share a bank
                        return ps[m][:, :w]

                    for k in range(KT):
                        for m in range(MT):
                            nc.tensor.matmul(
                                chain(m),
                                bq_tiles[k][:, m * P:(m + 1) * P],
                                w_slice(ci, k, woff, w),
                                start=(k == 0),
                                stop=(k == KT - 1),
                            )
                    for m in range(MT):
                        nc.vector.tensor_add(
                            qe[:, m * w:(m + 1) * w],
                            chain(m),
                            bias_t[:, lo:lo + w],
                        )
                    # qe -> ain: contiguous store, halved across the two
                    # HWDGE rings by partition band
                    qe3 = qe[:, :].rearrange("p (m c) -> p m c", m=MT)
                    for j in chunks:
                        off = j * NW - lo
                        nc.sync.dma_start(
                            out=ain[j][0:P // 2, :],
                            in_=qe3[0:P // 2, :, off:off + NW],
                        )
                        nc.scalar.dma_start(
                            out=ain[j][P // 2:P, :],
                            in_=qe3[P // 2:P, :, off:off + NW],
                        )
                    for j in chunks:
                        a2a(j)
                    # gathers for this pass's chunks: emitted here so they
                    # never queue behind a later pass's stores.  bq tokens
                    # are host-swizzled so this is one flat contiguous DMA.
                    for j in chunks:
                        q2 = q2pool.tile([P, TG * NW], DT_MM, name="q2")
                        nc.sync.dma_start(out=q2[:, :], in_=aout[j][:, :])
                        q2_tiles[j] = q2

            def stage2(j):
                with nc.named_scope(f"s2_q{j}"):
                    q2 = q2_tiles.pop(j)
                    for gp in range(TG // 2):
                        o2 = o2pool.tile([P, 2 * BG * NW], BF16, name="o2")
                        for gi in range(2):
                            g = gp * 2 + gi
                            # 4 batch-group matmuls -> two 1-bank PSUM
                            # tiles -> two half casts (DVE + ACT in
                            # parallel) for fine-grained slot recycling
                            for hb in range(2):
                                ps2 = pspool.tile([P, PSW], F32, name="ps")
                                for hh in range(2):
                                    h = hb * 2 + hh
                                    nc.tensor.matmul(
                                        ps2[:, hh * NW:(hh + 1) * NW],
                                        l_tiles[(g, h)][:, :],
                                        q2[:, g * NW:(g + 1) * NW],
                                        start=True,
                                        stop=True,
                                    )
                                dst = o2[:, gi * BG * NW + hb * 2 * NW:
                                         gi * BG * NW + (hb + 1) * 2 * NW]
                                if hb == 0:
                                    nc.vector.tensor_copy(dst, ps2[:, :])
                                else:
                                    nc.scalar.copy(dst, ps2[:, :])
                        if gp % 2 == 0:
                            nc.sync.dma_start(out=dq[j, gp], in_=o2[:, :])
                        else:
                            nc.scalar.dma_start(out=dq[j, gp], in_=o2[:, :])

            for pi in range(len(PASSES)):
                stage1_pass(pi)
                for j in S2_AFTER.get(pi, []):
                    stage2(j)

    nc.finalize()
    return nc


def _prep_inputs(query_weights, basic_queries, W_mlp, b_mlp):
    qw = np.ascontiguousarray(query_weights, dtype=np.float32)
    bq = np.ascontiguousarray(basic_queries, dtype=np.float32)
    W = np.ascontiguousarray(W_mlp, dtype=np.float32)
    b = np.ascontiguousarray(b_mlp, dtype=np.float32)

    # token swizzle v2: bqT column (mt*128 + c8*16 + u) holds token
    # (c8*128 + mt*16 + u), i.e. stage-1 M-tile mt = token group, qe
    # partition p = (dest core c8, in-group index u).  This makes the
    # qe->ain store contiguous, the a2a chunk boundary a partition band,
    # and aout exactly the stage-2 q2 layout.
    mt_i = np.arange(MT)[:, None, None]
    c8_i = np.arange(NC)[None, :, None]
    u_i = np.arange(16)[None, None, :]
    tok_idx = (c8_i * P + mt_i * 16 + u_i).reshape(-1)  # new-col -> old token
    bqT = np.ascontiguousarray(bq.T[:, tok_idx].astype(NP_MM))  # (D, T)

    g_i = np.arange(TG)[:, None, None, None, None]
    h_i = np.arange(BG)[None, :, None, None, None]
    tt_i = np.arange(16)[None, None, :, None, None]
    r_i = np.arange(R)[None, None, None, :, None]
    bb_i = np.arange(8)[None, None, None, None, :]

    in_maps = []
    for c in range(NC):
        Wc = np.ascontiguousarray(W[:, c * D:(c + 1) * D].astype(NP_MM))
        biasr = np.ascontiguousarray(
            np.broadcast_to(b[c * D:(c + 1) * D], (P, D))
        )
        qw_c = qw[:, c * TS:(c + 1) * TS, :]  # (32, 128, 8)
        # K index r*16+tt (r-major), M index bb*16+tt (b-major)
        L = np.zeros((TG, BG, P, P), NP_MM)
        L[g_i, h_i, r_i * 16 + tt_i, bb_i * 16 + tt_i] = \
            qw_c[h_i * 8 + bb_i, g_i * 16 + tt_i, r_i].astype(NP_MM)
        # pack to (128, 32*128): Lbig[p, (g*BG+h)*128 + m] = L[g, h, p, m]
        Lbig = np.ascontiguousarray(
            L.transpose(2, 0, 1, 3).reshape(P, TG * BG * P)
        )
        in_maps.append({"bqT": bqT, "Wc": Wc, "biasr": biasr, "Lt": Lbig})
    return in_maps


last_results = None  # exposed for external profiling harnesses


def kernel(query_weights, basic_queries, W_mlp, b_mlp):
    global last_results
    if "nc" not in _cache:
        _cache["nc"] = _build_nc()
    nc = _cache["nc"]

    in_maps = _prep_inputs(query_weights, basic_queries, W_mlp, b_mlp)
    res = run_bass_kernel_spmd(nc, in_maps, core_ids=list(range(NC)))
    last_results = res

    # dq[j, gp, (bb,tt), (gi,h,c)] -> (B, TS, D):  b = h*8+bb,
    # t = (2*gp+gi)*16+tt, d = j*NW+c
    parts = []
    for c in range(NC):
        arr = res.results[c]["dq"].reshape(NQ, TG // 2, 8, 16, 2, BG, NW)
        arr = arr.transpose(5, 2, 1, 4, 3, 0, 6).reshape(B, TS, D)
        parts.append(arr.astype(np.float32))
    dq_full = np.concatenate(parts, axis=1)
    basic_expanded = np.broadcast_to(
        np.ascontiguousarray(basic_queries, dtype=np.float32)[None], (B, T, D)
    )
    return dq_full, basic_expanded
